# revision 21
# baseline (speedup 1.0000x reference)
"""Single-head causal attention (B=8, T=2048, C=1024, H=64) on 8 TRN2 NeuronCores.

Strategy: pure data parallelism - batch element b runs on core b. Per core:

    Q = q_b @ Wq ; K = k_b @ Wk ; V = k_b @ Wv          (projections)
    S = Q @ K^T / sqrt(C), causal-masked ; P = exp(S)
    out_t = [(P @ V)^T ; P @ 1]   (unnormalized; host divides rows 0:64
                                   by row 64 and transposes)

Device-side choices (v3; evolved from 67us -> 57us -> this):
  * q ships as fp8(e4m3); Q projects with DoubleRow fp8 matmuls
    (contraction pairs interleaved): half the Q-proj TensorE cycles and
    2MB less HBM. k stays bf16 (V accuracy needs it).
  * K^T/V^T from one fused [Wk|Wv] projection. V natural tiles come from
    HWDGE DMA-transpose (xbar), entirely off TensorE; fp8 copies for PV.
  * S^T tiles (keys on partitions) in bf16; the two tiles of a pair run
    CONCURRENTLY in the PE via row groups (tile_position (0,0)/(64,0))
    and land in one 2-bank PSUM tile, so one ScalarE exp covers both.
  * Off-diagonal (fully causal-valid) tile pairs run P @ V as ONE fp8
    DoubleRow matmul (contraction 256); diagonal tiles stay bf16 with
    triangle masks. Softmax denominators ride a ones column (row 64).
  * No on-device normalization: host divides numerator rows by l.
  * 8 dummy matmuls on a zeroed tile right after the preamble warm the
    PE clock (HAM 4/8 -> 8/8) during otherwise-dead DMA-wait time.
  * DMA: inputs on the sync HWDGE ring (k0 split fine so the first
    matmul starts ASAP; then q0, k1, q1, ...); weights on the scalar
    ring; mask consts on gpsimd SWDGE; V-transposes + output stores on
    the sync ring behind the inputs. Emission weaves next-block
    projection matmuls into the exp-paced back half of each attention
    block.
"""

import numpy as np
import ml_dtypes

B, T, C, H = 8, 2048, 1024, 64
P = 128                  # SBUF partitions
CCH = C // P             # 8 contraction chunks
G = CCH // 2             # 4 DoubleRow chunk pairs
NJ = T // P              # 16 key tiles of 128
NB = T // 512            # 4 column blocks of 512
SCALE = float(C) ** -0.5

FP8Q = True              # q fp8 + DoubleRow Q projection
FP8PV = True             # fp8 DoubleRow P@V for off-diagonal pairs
SPAIR = True             # row-paired (concurrent) S matmuls (blocks >= 1)
SPAIR_DUP = True         # operands for row group h1 from duplicated tiles
VT_DMA = False           # V transposes via DMA xbar instead of TensorE
WARMUP = 8               # dummy matmuls to warm the PE clock

_cached = {}


def _build(fp8q=FP8Q, fp8pv=FP8PV, spair=SPAIR, spair_dup=SPAIR_DUP,
           vt_dma=VT_DMA, warmup=WARMUP):
    import concourse.bass as bass
    import concourse.mybir as mybir
    import concourse.tile as tile
    from concourse import bacc

    dt = mybir.dt
    nc = bacc.Bacc("TRN2", target_bir_lowering=False, debug=False, num_devices=B)

    kT = nc.dram_tensor("kT", [NB, P, CCH, 512], dt.bfloat16, kind="ExternalInput").ap()
    if fp8q:
        qT = nc.dram_tensor("qT", [NB, P, G, 2, 512], dt.float8e4,
                            kind="ExternalInput").ap()
        wq = nc.dram_tensor("wq", [P, G, 2, H], dt.float8e4,
                            kind="ExternalInput").ap()
    else:
        qT = nc.dram_tensor("qT", [NB, P, CCH, 512], dt.bfloat16,
                            kind="ExternalInput").ap()
        wq = nc.dram_tensor("wq", [P, CCH, H], dt.bfloat16,
                            kind="ExternalInput").ap()
    wkv = nc.dram_tensor("wkv", [P, CCH, 2 * H], dt.bfloat16, kind="ExternalInput").ap()
    dmask = nc.dram_tensor("dmask", [P, P], dt.bfloat16, kind="ExternalInput").ap()
    idb = nc.dram_tensor("idb", [P, P], dt.bfloat16, kind="ExternalInput").ap()
    out_t = nc.dram_tensor("out_t", [H + 1, T], dt.float32, kind="ExternalOutput").ap()

    EXP = mybir.ActivationFunctionType.Exp
    DR = mybir.MatmulPerfMode.DoubleRow

    with tile.TileContext(nc) as tc:
        with (
            tc.tile_pool(name="consts", bufs=1) as consts,
            tc.tile_pool(name="inbuf", bufs=1) as inbuf,
            tc.tile_pool(name="proj", bufs=1) as proj,
            tc.tile_pool(name="ppsum", bufs=1, space="PSUM") as ppsum,
            tc.tile_pool(name="opsum", bufs=2 if vt_dma else 1,
                         space="PSUM") as opsum,
            tc.tile_pool(name="spsum", bufs=2, space="PSUM") as spsum,
            tc.tile_pool(name="vtpsum", bufs=1, space="PSUM") as vtpsum,
            tc.tile_pool(name="pbuf", bufs=2) as pbuf,
            tc.tile_pool(name="obuf", bufs=2) as obuf,
        ):

            # ---- constants: weights on scalar HWDGE ring (earliest need),
            #      mask/identity on the gpsimd SWDGE ring -------------------
            wkv_s = consts.tile([P, CCH, 2 * H], dt.bfloat16)
            if fp8q:
                wq_s = consts.tile([P, G, 2, H], dt.float8e4)
            else:
                wq_s = consts.tile([P, CCH, H], dt.bfloat16)
            mask_s = consts.tile([P, P], dt.bfloat16)
            idb_s = consts.tile([P, P], dt.bfloat16)
            nc.sync.dma_start(out=wkv_s[:], in_=wkv[:])
            nc.sync.dma_start(out=wq_s[:], in_=wq[:])
            nc.scalar.dma_start(out=mask_s[:], in_=dmask[:])
            nc.scalar.dma_start(out=idb_s[:], in_=idb[:])

            # ---- PE clock warm-up on a zeroed tile (no DMA dependence) ----
            if warmup:
                warm_s = consts.tile([P, 512], dt.bfloat16)
                nc.vector.memset(warm_s[:], 0.0)
                wt = spsum.tile([P, 2, 512], dt.float32, tag="s")
                for _ in range(warmup):
                    nc.tensor.matmul(wt[:, 0, :], lhsT=warm_s[:, 0:P],
                                     rhs=warm_s[:], start=True, stop=True)

            kT_s = inbuf.tile([P, NB, CCH, 512], dt.bfloat16)
            if fp8q:
                qT_s = inbuf.tile([P, NB, G, 2, 512], dt.float8e4)
            else:
                qT_s = inbuf.tile([P, NB, CCH, 512], dt.bfloat16)
            KVT_s = proj.tile([P, T], dt.bfloat16)   # rows 0:64 K^T, 64:128 V^T
            QT_s = proj.tile([H, T], dt.bfloat16)
            if spair and spair_dup:
                KT2_s = proj.tile([P, T], dt.bfloat16)   # K^T dup @ rows 64:128
                QT2_s = proj.tile([P, T], dt.bfloat16)   # Q^T dup @ rows 64:128
            # V natural tiles at [:, j, 0:64], ones column at [:, j, 64]
            V1_s = proj.tile([P, NJ, 80], dt.bfloat16)
            nc.vector.memset(V1_s[:, :, 64:65], 1.0)
            if fp8pv:
                V18_s = proj.tile([P, NJ, 80], dt.float8e4)
                nc.vector.memset(V18_s[:, :, 64:65], 1.0)

            # ---- input DMAs: k blocks on the sync ring, q blocks on the
            #      scalar ring - the two HWDGE rings overlap each other's
            #      per-transfer completion latency ---------------------------
            for tb in range(NB):
                nc.sync.dma_start(out=kT_s[:, tb], in_=kT[tb])
            for tb in range(NB):
                nc.scalar.dma_start(out=qT_s[:, tb], in_=qT[tb])

            # ---- projection work-units for one 512-col block ---------------
            def proj_units(tb, first=False):
                """Returns (core_units, vt_units)."""
                sl = slice(512 * tb, 512 * (tb + 1))
                KVTp = ppsum.tile([P, 512], dt.float32, tag="kvt")
                QTp = ppsum.tile([H, 512], dt.float32, tag="qt")
                core = []

                def kv_unit(cs, KVTp=KVTp):
                    for c in cs:
                        nc.tensor.matmul(KVTp[:], lhsT=wkv_s[:, c, :],
                                         rhs=kT_s[:, tb, c, :],
                                         start=(c == 0), stop=(c == CCH - 1))
                groups = ([0, 1], [2, 3], [4, 5], [6, 7])
                for cs in groups:
                    core.append(lambda cs=cs: kv_unit(cs))

                def kv_copy(KVTp=KVTp):
                    nc.vector.tensor_copy(out=KVT_s[:, sl], in_=KVTp[:])
                    if spair and spair_dup:
                        nc.gpsimd.dma_start(out=KT2_s[64:128, sl],
                                            in_=KVT_s[0:64, sl])
                core.append(kv_copy)

                if fp8q:
                    def q_unit(QTp=QTp):
                        for g in range(G):
                            nc.tensor.matmul(QTp[:], lhsT=wq_s[:, g],
                                             rhs=qT_s[:, tb, g],
                                             start=(g == 0), stop=(g == G - 1),
                                             perf_mode=DR)
                    core.append(q_unit)
                else:
                    def q_unit(cc, QTp=QTp):
                        for c in (cc, cc + 1):
                            nc.tensor.matmul(QTp[:], lhsT=wq_s[:, c, :],
                                             rhs=qT_s[:, tb, c, :],
                                             start=(c == 0), stop=(c == CCH - 1))
                    for cc in range(0, CCH, 2):
                        core.append(lambda cc=cc: q_unit(cc))

                def q_copy(QTp=QTp):
                    nc.vector.tensor_copy(out=QT_s[:, sl], in_=QTp[:])
                    if spair and spair_dup and tb >= 1:
                        # (block 0 attends unpaired - no dup needed there)
                        nc.gpsimd.dma_start(out=QT2_s[64:128, sl],
                                            in_=QT_s[0:H, sl])
                core.append(q_copy)

                def vt_unit(j):
                    if vt_dma:
                        nc.sync.dma_start_transpose(
                            out=V1_s[:, j, 0:64],
                            in_=KVT_s[64:128, P * j:P * (j + 1)])
                        if fp8pv:
                            nc.vector.tensor_copy(out=V18_s[:, j, 0:64],
                                                  in_=V1_s[:, j, 0:64])
                    else:
                        vtp = vtpsum.tile([P, P], dt.bfloat16, tag="vt")
                        nc.tensor.transpose(vtp[:], KVT_s[:, P * j:P * (j + 1)],
                                            idb_s[:])
                        nc.vector.tensor_copy(out=V1_s[:, j, 0:64],
                                              in_=vtp[:, 64:128])
                        if fp8pv:
                            nc.vector.tensor_copy(out=V18_s[:, j, 0:64],
                                                  in_=vtp[:, 64:128])
                vt = [lambda j=4 * tb + jj: vt_unit(j) for jj in range(4)]
                return core, vt

            # ---- attention block: pairs of 128-key tiles -------------------
            def attn_block(ic, weave, early):
                ilo = 512 * ic
                OUTp = opsum.tile([H + 1, 512], dt.float32, tag="out")
                pairs = []
                for u in range(2 * ic):
                    pairs.append((2 * u, 512, 2 * u + 1, 512, False))
                pairs.append((4 * ic, 512, 4 * ic + 1, 384, True))
                pairs.append((4 * ic + 2, 256, 4 * ic + 3, 128, True))
                npairs = len(pairs)
                state = {}
                pair_ok = spair and spair_dup and ic >= 1

                def emit_S(pr):
                    jA, nA, jB, nB, diag = pr
                    Sp = spsum.tile([P, 2, 512], dt.float32, tag="s")
                    loA = max(P * jA, ilo)
                    loB = max(P * jB, ilo)
                    nc.tensor.matmul(Sp[:, 0, 0:nA],
                                     lhsT=KVT_s[0:H, P * jA:P * (jA + 1)],
                                     rhs=QT_s[:, loA:loA + nA],
                                     start=True, stop=True)
                    if pair_ok:
                        # both operands must physically sit at partitions
                        # 64:128 (walrus: weight base == row tile position,
                        # fmap base == weight base) - hence the dup tiles.
                        nc.tensor.matmul(Sp[:, 1, 0:nB],
                                         lhsT=KT2_s[64:128, P * jB:P * (jB + 1)],
                                         rhs=QT2_s[64:128, loB:loB + nB],
                                         start=True, stop=True,
                                         tile_position=(64, 0))
                    else:
                        nc.tensor.matmul(Sp[:, 1, 0:nB],
                                         lhsT=KVT_s[0:H, P * jB:P * (jB + 1)],
                                         rhs=QT_s[:, loB:loB + nB],
                                         start=True, stop=True)
                    if diag or not fp8pv:
                        Pt = pbuf.tile([P, 2, 512], dt.bfloat16, tag="pd", bufs=2)
                    else:
                        Pt = pbuf.tile([P, 2, 512], dt.float8e4, tag="pf", bufs=3)
                    nc.scalar.activation(out=Pt[:, :, 0:nA], in_=Sp[:, :, 0:nA],
                                         func=EXP, scale=SCALE)
                    if diag:
                        nc.vector.tensor_mul(Pt[:, 0, 0:P], Pt[:, 0, 0:P],
                                             mask_s[:])
                        nc.vector.tensor_mul(Pt[:, 1, 0:P], Pt[:, 1, 0:P],
                                             mask_s[:])
                    state[pr[0]] = Pt

                def emit_PV(pr, first, last):
                    jA, nA, jB, nB, diag = pr
                    Pt = state.pop(pr[0])
                    if diag or not fp8pv:
                        loA = max(P * jA, ilo) - ilo
                        loB = max(P * jB, ilo) - ilo
                        nc.tensor.matmul(OUTp[:, loA:512],
                                         lhsT=V1_s[:, jA, 0:65],
                                         rhs=Pt[:, 0, 0:nA],
                                         start=first, stop=False)
                        nc.tensor.matmul(OUTp[:, loB:512],
                                         lhsT=V1_s[:, jB, 0:65],
                                         rhs=Pt[:, 1, 0:nB],
                                         start=False, stop=last)
                    else:
                        nc.tensor.matmul(OUTp[:, 0:512],
                                         lhsT=V18_s[:, jA:jA + 2, 0:65],
                                         rhs=Pt[:, :, :],
                                         start=first, stop=last,
                                         perf_mode=DR)

                # software pipeline: S(p+1) emitted before PV(p); `early`
                # units go right after S(0); `weave` into the back half.
                widx = 0
                wstart = (npairs + 1) // 2

                def drain(tgt):
                    nonlocal widx
                    while widx < min(tgt, len(weave)):
                        weave[widx]()
                        widx += 1

                emit_S(pairs[0])
                for u in early:
                    u()
                nslots = npairs - wstart
                for i in range(1, npairs):
                    emit_S(pairs[i])
                    emit_PV(pairs[i - 1], first=(i == 1), last=False)
                    if i >= wstart and nslots > 0:
                        drain((i - wstart + 1) * len(weave) // (nslots + 1))
                emit_PV(pairs[-1], first=(npairs == 1), last=True)
                drain(len(weave))

                ot = obuf.tile([H + 1, 512], dt.float32, tag="o")
                nc.vector.tensor_copy(out=ot[:], in_=OUTp[:])
                nc.sync.dma_start(out=out_t[:, ilo:ilo + 512], in_=ot[:])

            core0, vt0 = proj_units(0, first=True)
            for u in core0:
                u()
            nxt = vt0
            for ic in range(NB):
                if ic + 1 < NB:
                    core, vt = proj_units(ic + 1)
                    weave = core + vt
                else:
                    weave = []
                attn_block(ic, weave, early=nxt)
                nxt = []

    nc.compile()
    return nc


def _get_nc():
    key = (FP8Q, FP8PV, SPAIR, SPAIR_DUP, VT_DMA, WARMUP)
    if key not in _cached:
        _cached[key] = _build(*key)
    return _cached[key]


def _block(xT):
    """[C, T] -> [NB, P, CCH, 512] so each 512-col block is contiguous."""
    return np.ascontiguousarray(
        xT.reshape(CCH, P, NB, 512).transpose(2, 1, 0, 3))


def _block8(xT):
    """[C, T] -> [NB, P, G, 2, 512]; contraction chunk pairs interleaved."""
    return np.ascontiguousarray(
        xT.reshape(G, 2, P, NB, 512).transpose(3, 2, 0, 1, 4))


def _wblock(w):
    """[C, Hw] -> [P, CCH, Hw] contiguous."""
    return np.ascontiguousarray(
        w.reshape(CCH, P, w.shape[1]).transpose(1, 0, 2))


def _host_inputs(q, k, Wq, Wk, Wv):
    bf16 = ml_dtypes.bfloat16
    fp8 = ml_dtypes.float8_e4m3
    if FP8Q:
        wq_h = np.ascontiguousarray(
            Wq.astype(fp8).reshape(G, 2, P, H).transpose(2, 0, 1, 3))
    else:
        wq_h = _wblock(Wq.astype(bf16))
    wkv_h = _wblock(np.concatenate([Wk, Wv], axis=1).astype(bf16))
    dmask_h = np.triu(np.ones((P, P), dtype=np.float32)).astype(bf16)
    idb_h = np.eye(P, dtype=np.float32).astype(bf16)
    in_maps = []
    for b in range(B):
        in_maps.append({
            "qT": _block8(q[b].T.astype(fp8)) if FP8Q
                  else _block(q[b].T.astype(bf16)),
            "kT": _block(k[b].T.astype(bf16)),
            "wq": wq_h,
            "wkv": wkv_h,
            "dmask": dmask_h,
            "idb": idb_h,
        })
    return in_maps


def _unshard(results):
    """Per-core [H+1, T] (numerator^T ; l) -> [B, T, H] normalized fp32."""
    outs = []
    for b in range(B):
        ot = results[b]["out_t"].astype(np.float32)
        outs.append((ot[0:H] / ot[H:H + 1]).T)
    return np.stack(outs).astype(np.float32)


def kernel(q, k, Wq, Wk, Wv):
    from concourse.bass_utils import run_bass_kernel_spmd

    nc = _get_nc()
    in_maps = _host_inputs(q, k, Wq, Wk, Wv)
    res = run_bass_kernel_spmd(nc, in_maps, list(range(B)))
    return _unshard(res.results)


if __name__ == "__main__":
    rng = np.random.default_rng(0)
    q = rng.standard_normal((B, T, C)).astype(np.float32)
    k = rng.standard_normal((B, T, C)).astype(np.float32)
    Wq = (rng.standard_normal((C, H)) * 0.02).astype(np.float32)
    Wk = (rng.standard_normal((C, H)) * 0.02).astype(np.float32)
    Wv = (rng.standard_normal((C, H)) * 0.02).astype(np.float32)
    o = kernel(q, k, Wq, Wk, Wv)
    print("out", o.shape, o.dtype, float(np.abs(o).max()))


# revision 23
# speedup vs baseline: 1.0019x; 1.0019x over previous
"""Single-head causal attention (B=8, T=2048, C=1024, H=64) on 8 TRN2 NeuronCores.

Strategy: pure data parallelism - batch element b runs on core b. Per core:

    Q = q_b @ Wq ; K = k_b @ Wk ; V = k_b @ Wv          (projections)
    S = Q @ K^T / sqrt(C), causal-masked ; P = exp(S)
    out_t = [(P @ V)^T ; P @ 1]   (unnormalized; host divides rows 0:64
                                   by row 64 and transposes)

Device-side choices (v3; evolved from 67us -> 57us -> this):
  * q ships as fp8(e4m3); Q projects with DoubleRow fp8 matmuls
    (contraction pairs interleaved): half the Q-proj TensorE cycles and
    2MB less HBM. k stays bf16 (V accuracy needs it).
  * K^T/V^T from one fused [Wk|Wv] projection. V natural tiles come from
    HWDGE DMA-transpose (xbar), entirely off TensorE; fp8 copies for PV.
  * S^T tiles (keys on partitions) in bf16; the two tiles of a pair run
    CONCURRENTLY in the PE via row groups (tile_position (0,0)/(64,0))
    and land in one 2-bank PSUM tile, so one ScalarE exp covers both.
  * Off-diagonal (fully causal-valid) tile pairs run P @ V as ONE fp8
    DoubleRow matmul (contraction 256); diagonal tiles stay bf16 with
    triangle masks. Softmax denominators ride a ones column (row 64).
  * No on-device normalization: host divides numerator rows by l.
  * 8 dummy matmuls on a zeroed tile right after the preamble warm the
    PE clock (HAM 4/8 -> 8/8) during otherwise-dead DMA-wait time.
  * DMA: inputs on the sync HWDGE ring (k0 split fine so the first
    matmul starts ASAP; then q0, k1, q1, ...); weights on the scalar
    ring; mask consts on gpsimd SWDGE; V-transposes + output stores on
    the sync ring behind the inputs. Emission weaves next-block
    projection matmuls into the exp-paced back half of each attention
    block.
"""

import numpy as np
import ml_dtypes

B, T, C, H = 8, 2048, 1024, 64
P = 128                  # SBUF partitions
CCH = C // P             # 8 contraction chunks
G = CCH // 2             # 4 DoubleRow chunk pairs
NJ = T // P              # 16 key tiles of 128
NB = T // 512            # 4 column blocks of 512
SCALE = float(C) ** -0.5

FP8Q = True              # q fp8 + DoubleRow Q projection
FP8PV = True             # fp8 DoubleRow P@V for off-diagonal pairs
SPAIR = True             # row-paired (concurrent) S matmuls (blocks >= 1)
SPAIR_DUP = True         # operands for row group h1 from duplicated tiles
VT_DMA = False           # V transposes via DMA xbar instead of TensorE
WARMUP = 8               # dummy matmuls to warm the PE clock

_cached = {}


def _build(fp8q=FP8Q, fp8pv=FP8PV, spair=SPAIR, spair_dup=SPAIR_DUP,
           vt_dma=VT_DMA, warmup=WARMUP):
    import concourse.bass as bass
    import concourse.mybir as mybir
    import concourse.tile as tile
    from concourse import bacc

    dt = mybir.dt
    nc = bacc.Bacc("TRN2", target_bir_lowering=False, debug=False, num_devices=B)

    kT = nc.dram_tensor("kT", [NB, P, CCH, 512], dt.bfloat16, kind="ExternalInput").ap()
    if fp8q:
        qT = nc.dram_tensor("qT", [NB, P, G, 2, 512], dt.float8e4,
                            kind="ExternalInput").ap()
        wq = nc.dram_tensor("wq", [P, G, 2, H], dt.float8e4,
                            kind="ExternalInput").ap()
    else:
        qT = nc.dram_tensor("qT", [NB, P, CCH, 512], dt.bfloat16,
                            kind="ExternalInput").ap()
        wq = nc.dram_tensor("wq", [P, CCH, H], dt.bfloat16,
                            kind="ExternalInput").ap()
    wkv = nc.dram_tensor("wkv", [P, CCH, 2 * H], dt.bfloat16, kind="ExternalInput").ap()
    dmask = nc.dram_tensor("dmask", [P, P], dt.bfloat16, kind="ExternalInput").ap()
    idb = nc.dram_tensor("idb", [P, P], dt.bfloat16, kind="ExternalInput").ap()
    out_t = nc.dram_tensor("out_t", [H + 1, T], dt.float32, kind="ExternalOutput").ap()

    EXP = mybir.ActivationFunctionType.Exp
    DR = mybir.MatmulPerfMode.DoubleRow

    with tile.TileContext(nc) as tc:
        with (
            tc.tile_pool(name="consts", bufs=1) as consts,
            tc.tile_pool(name="inbuf", bufs=1) as inbuf,
            tc.tile_pool(name="proj", bufs=1) as proj,
            tc.tile_pool(name="ppsum", bufs=1, space="PSUM") as ppsum,
            tc.tile_pool(name="opsum", bufs=2 if vt_dma else 1,
                         space="PSUM") as opsum,
            tc.tile_pool(name="spsum", bufs=2, space="PSUM") as spsum,
            tc.tile_pool(name="vtpsum", bufs=1, space="PSUM") as vtpsum,
            tc.tile_pool(name="pbuf", bufs=2) as pbuf,
            tc.tile_pool(name="obuf", bufs=2) as obuf,
        ):

            # ---- constants: weights on scalar HWDGE ring (earliest need),
            #      mask/identity on the gpsimd SWDGE ring -------------------
            wkv_s = consts.tile([P, CCH, 2 * H], dt.bfloat16)
            if fp8q:
                wq_s = consts.tile([P, G, 2, H], dt.float8e4)
            else:
                wq_s = consts.tile([P, CCH, H], dt.bfloat16)
            mask_s = consts.tile([P, P], dt.bfloat16)
            idb_s = consts.tile([P, P], dt.bfloat16)
            # weights lead the sync ring (the data ring is FIFO: putting
            # everything in strict need-order beats parallel rings, whose
            # transfers fair-share SDMA bandwidth and all finish late)
            nc.sync.dma_start(out=wkv_s[:], in_=wkv[:])
            nc.sync.dma_start(out=wq_s[:], in_=wq[:])
            nc.scalar.dma_start(out=mask_s[:], in_=dmask[:])
            nc.scalar.dma_start(out=idb_s[:], in_=idb[:])

            # ---- PE clock warm-up on a zeroed tile (no DMA dependence) ----
            if warmup:
                warm_s = consts.tile([P, 512], dt.bfloat16)
                nc.vector.memset(warm_s[:], 0.0)
                wt = spsum.tile([P, 2, 512], dt.float32, tag="s")
                for _ in range(warmup):
                    nc.tensor.matmul(wt[:, 0, :], lhsT=warm_s[:, 0:P],
                                     rhs=warm_s[:], start=True, stop=True)

            kT_s = inbuf.tile([P, NB, CCH, 512], dt.bfloat16)
            if fp8q:
                qT_s = inbuf.tile([P, NB, G, 2, 512], dt.float8e4)
            else:
                qT_s = inbuf.tile([P, NB, CCH, 512], dt.bfloat16)
            KVT_s = proj.tile([P, T], dt.bfloat16)   # rows 0:64 K^T, 64:128 V^T
            QT_s = proj.tile([H, T], dt.bfloat16)
            if spair and spair_dup:
                KT2_s = proj.tile([P, T], dt.bfloat16)   # K^T dup @ rows 64:128
                QT2_s = proj.tile([P, T], dt.bfloat16)   # Q^T dup @ rows 64:128
            # V natural tiles at [:, j, 0:64], ones column at [:, j, 64]
            V1_s = proj.tile([P, NJ, 80], dt.bfloat16)
            nc.vector.memset(V1_s[:, :, 64:65], 1.0)
            if fp8pv:
                V18_s = proj.tile([P, NJ, 80], dt.float8e4)
                nc.vector.memset(V18_s[:, :, 64:65], 1.0)

            # ---- input DMAs: one FIFO ring, strict need-order ---------------
            for tb in range(NB):
                nc.sync.dma_start(out=kT_s[:, tb], in_=kT[tb])
                nc.sync.dma_start(out=qT_s[:, tb], in_=qT[tb])

            # ---- projection work-units for one 512-col block ---------------
            def proj_units(tb, first=False):
                """Returns (core_units, vt_units)."""
                sl = slice(512 * tb, 512 * (tb + 1))
                KVTp = ppsum.tile([P, 512], dt.float32, tag="kvt")
                QTp = ppsum.tile([H, 512], dt.float32, tag="qt")
                core = []

                def kv_unit(cs, KVTp=KVTp):
                    for c in cs:
                        nc.tensor.matmul(KVTp[:], lhsT=wkv_s[:, c, :],
                                         rhs=kT_s[:, tb, c, :],
                                         start=(c == 0), stop=(c == CCH - 1))
                groups = ([0, 1], [2, 3], [4, 5], [6, 7])
                for cs in groups:
                    core.append(lambda cs=cs: kv_unit(cs))

                def kv_copy(KVTp=KVTp):
                    nc.vector.tensor_copy(out=KVT_s[:, sl], in_=KVTp[:])
                    if spair and spair_dup:
                        nc.gpsimd.dma_start(out=KT2_s[64:128, sl],
                                            in_=KVT_s[0:64, sl])
                core.append(kv_copy)

                if fp8q:
                    def q_unit(QTp=QTp):
                        for g in range(G):
                            nc.tensor.matmul(QTp[:], lhsT=wq_s[:, g],
                                             rhs=qT_s[:, tb, g],
                                             start=(g == 0), stop=(g == G - 1),
                                             perf_mode=DR)
                    core.append(q_unit)
                else:
                    def q_unit(cc, QTp=QTp):
                        for c in (cc, cc + 1):
                            nc.tensor.matmul(QTp[:], lhsT=wq_s[:, c, :],
                                             rhs=qT_s[:, tb, c, :],
                                             start=(c == 0), stop=(c == CCH - 1))
                    for cc in range(0, CCH, 2):
                        core.append(lambda cc=cc: q_unit(cc))

                def q_copy(QTp=QTp):
                    nc.vector.tensor_copy(out=QT_s[:, sl], in_=QTp[:])
                    if spair and spair_dup and tb >= 1:
                        # (block 0 attends unpaired - no dup needed there)
                        nc.gpsimd.dma_start(out=QT2_s[64:128, sl],
                                            in_=QT_s[0:H, sl])
                core.append(q_copy)

                def vt_unit(j):
                    if vt_dma:
                        nc.sync.dma_start_transpose(
                            out=V1_s[:, j, 0:64],
                            in_=KVT_s[64:128, P * j:P * (j + 1)])
                        if fp8pv:
                            nc.vector.tensor_copy(out=V18_s[:, j, 0:64],
                                                  in_=V1_s[:, j, 0:64])
                    else:
                        vtp = vtpsum.tile([P, P], dt.bfloat16, tag="vt")
                        nc.tensor.transpose(vtp[:], KVT_s[:, P * j:P * (j + 1)],
                                            idb_s[:])
                        nc.vector.tensor_copy(out=V1_s[:, j, 0:64],
                                              in_=vtp[:, 64:128])
                        if fp8pv:
                            nc.vector.tensor_copy(out=V18_s[:, j, 0:64],
                                                  in_=vtp[:, 64:128])
                vt = [lambda j=4 * tb + jj: vt_unit(j) for jj in range(4)]
                return core, vt

            # ---- attention block: pairs of 128-key tiles -------------------
            def attn_block(ic, weave, early):
                ilo = 512 * ic
                OUTp = opsum.tile([H + 1, 512], dt.float32, tag="out")
                pairs = []
                for u in range(2 * ic):
                    pairs.append((2 * u, 512, 2 * u + 1, 512, False))
                pairs.append((4 * ic, 512, 4 * ic + 1, 384, True))
                pairs.append((4 * ic + 2, 256, 4 * ic + 3, 128, True))
                npairs = len(pairs)
                state = {}
                pair_ok = spair and spair_dup and ic >= 1

                def emit_S(pr):
                    jA, nA, jB, nB, diag = pr
                    Sp = spsum.tile([P, 2, 512], dt.float32, tag="s")
                    loA = max(P * jA, ilo)
                    loB = max(P * jB, ilo)
                    nc.tensor.matmul(Sp[:, 0, 0:nA],
                                     lhsT=KVT_s[0:H, P * jA:P * (jA + 1)],
                                     rhs=QT_s[:, loA:loA + nA],
                                     start=True, stop=True)
                    if pair_ok:
                        # both operands must physically sit at partitions
                        # 64:128 (walrus: weight base == row tile position,
                        # fmap base == weight base) - hence the dup tiles.
                        nc.tensor.matmul(Sp[:, 1, 0:nB],
                                         lhsT=KT2_s[64:128, P * jB:P * (jB + 1)],
                                         rhs=QT2_s[64:128, loB:loB + nB],
                                         start=True, stop=True,
                                         tile_position=(64, 0))
                    else:
                        nc.tensor.matmul(Sp[:, 1, 0:nB],
                                         lhsT=KVT_s[0:H, P * jB:P * (jB + 1)],
                                         rhs=QT_s[:, loB:loB + nB],
                                         start=True, stop=True)
                    if diag or not fp8pv:
                        Pt = pbuf.tile([P, 2, 512], dt.bfloat16, tag="pd", bufs=2)
                    else:
                        Pt = pbuf.tile([P, 2, 512], dt.float8e4, tag="pf", bufs=3)
                    nc.scalar.activation(out=Pt[:, :, 0:nA], in_=Sp[:, :, 0:nA],
                                         func=EXP, scale=SCALE)
                    if diag:
                        nc.vector.tensor_mul(Pt[:, 0, 0:P], Pt[:, 0, 0:P],
                                             mask_s[:])
                        nc.vector.tensor_mul(Pt[:, 1, 0:P], Pt[:, 1, 0:P],
                                             mask_s[:])
                    state[pr[0]] = Pt

                def emit_PV(pr, first, last):
                    jA, nA, jB, nB, diag = pr
                    Pt = state.pop(pr[0])
                    if diag or not fp8pv:
                        loA = max(P * jA, ilo) - ilo
                        loB = max(P * jB, ilo) - ilo
                        nc.tensor.matmul(OUTp[:, loA:512],
                                         lhsT=V1_s[:, jA, 0:65],
                                         rhs=Pt[:, 0, 0:nA],
                                         start=first, stop=False)
                        nc.tensor.matmul(OUTp[:, loB:512],
                                         lhsT=V1_s[:, jB, 0:65],
                                         rhs=Pt[:, 1, 0:nB],
                                         start=False, stop=last)
                    else:
                        nc.tensor.matmul(OUTp[:, 0:512],
                                         lhsT=V18_s[:, jA:jA + 2, 0:65],
                                         rhs=Pt[:, :, :],
                                         start=first, stop=last,
                                         perf_mode=DR)

                # software pipeline: S(p+1) emitted before PV(p); `early`
                # units go right after S(0); `weave` into the back half.
                widx = 0
                wstart = (npairs + 1) // 2

                def drain(tgt):
                    nonlocal widx
                    while widx < min(tgt, len(weave)):
                        weave[widx]()
                        widx += 1

                emit_S(pairs[0])
                for u in early:
                    u()
                nslots = npairs - wstart
                for i in range(1, npairs):
                    emit_S(pairs[i])
                    emit_PV(pairs[i - 1], first=(i == 1), last=False)
                    if i >= wstart and nslots > 0:
                        drain((i - wstart + 1) * len(weave) // (nslots + 1))
                emit_PV(pairs[-1], first=(npairs == 1), last=True)
                drain(len(weave))

                ot = obuf.tile([H + 1, 512], dt.float32, tag="o")
                nc.vector.tensor_copy(out=ot[:], in_=OUTp[:])
                nc.sync.dma_start(out=out_t[:, ilo:ilo + 512], in_=ot[:])

            core0, vt0 = proj_units(0, first=True)
            for u in core0:
                u()
            nxt = vt0
            for ic in range(NB):
                if ic + 1 < NB:
                    core, vt = proj_units(ic + 1)
                    weave = core + vt
                else:
                    weave = []
                attn_block(ic, weave, early=nxt)
                nxt = []

    nc.compile()
    return nc


def _get_nc():
    key = (FP8Q, FP8PV, SPAIR, SPAIR_DUP, VT_DMA, WARMUP)
    if key not in _cached:
        _cached[key] = _build(*key)
    return _cached[key]


def _block(xT):
    """[C, T] -> [NB, P, CCH, 512] so each 512-col block is contiguous."""
    return np.ascontiguousarray(
        xT.reshape(CCH, P, NB, 512).transpose(2, 1, 0, 3))


def _block8(xT):
    """[C, T] -> [NB, P, G, 2, 512]; contraction chunk pairs interleaved."""
    return np.ascontiguousarray(
        xT.reshape(G, 2, P, NB, 512).transpose(3, 2, 0, 1, 4))


def _wblock(w):
    """[C, Hw] -> [P, CCH, Hw] contiguous."""
    return np.ascontiguousarray(
        w.reshape(CCH, P, w.shape[1]).transpose(1, 0, 2))


def _host_inputs(q, k, Wq, Wk, Wv):
    bf16 = ml_dtypes.bfloat16
    fp8 = ml_dtypes.float8_e4m3
    if FP8Q:
        wq_h = np.ascontiguousarray(
            Wq.astype(fp8).reshape(G, 2, P, H).transpose(2, 0, 1, 3))
    else:
        wq_h = _wblock(Wq.astype(bf16))
    wkv_h = _wblock(np.concatenate([Wk, Wv], axis=1).astype(bf16))
    dmask_h = np.triu(np.ones((P, P), dtype=np.float32)).astype(bf16)
    idb_h = np.eye(P, dtype=np.float32).astype(bf16)
    in_maps = []
    for b in range(B):
        in_maps.append({
            "qT": _block8(q[b].T.astype(fp8)) if FP8Q
                  else _block(q[b].T.astype(bf16)),
            "kT": _block(k[b].T.astype(bf16)),
            "wq": wq_h,
            "wkv": wkv_h,
            "dmask": dmask_h,
            "idb": idb_h,
        })
    return in_maps


def _unshard(results):
    """Per-core [H+1, T] (numerator^T ; l) -> [B, T, H] normalized fp32."""
    outs = []
    for b in range(B):
        ot = results[b]["out_t"].astype(np.float32)
        outs.append((ot[0:H] / ot[H:H + 1]).T)
    return np.stack(outs).astype(np.float32)


def kernel(q, k, Wq, Wk, Wv):
    from concourse.bass_utils import run_bass_kernel_spmd

    nc = _get_nc()
    in_maps = _host_inputs(q, k, Wq, Wk, Wv)
    res = run_bass_kernel_spmd(nc, in_maps, list(range(B)))
    return _unshard(res.results)


if __name__ == "__main__":
    rng = np.random.default_rng(0)
    q = rng.standard_normal((B, T, C)).astype(np.float32)
    k = rng.standard_normal((B, T, C)).astype(np.float32)
    Wq = (rng.standard_normal((C, H)) * 0.02).astype(np.float32)
    Wk = (rng.standard_normal((C, H)) * 0.02).astype(np.float32)
    Wv = (rng.standard_normal((C, H)) * 0.02).astype(np.float32)
    o = kernel(q, k, Wq, Wk, Wv)
    print("out", o.shape, o.dtype, float(np.abs(o).max()))


# revision 24
# speedup vs baseline: 1.1107x; 1.1087x over previous
"""Single-head causal attention (B=8, T=2048, C=1024, H=64) on 8 TRN2 NeuronCores.

Strategy: pure data parallelism - batch element b runs on core b. Per core:

    Q = q_b @ Wq ; K = k_b @ Wk ; V = k_b @ Wv          (projections)
    S = Q @ K^T / sqrt(C), causal-masked ; P = exp(S)
    out_t = [(P @ V)^T ; P @ 1]   (unnormalized; host divides rows 0:64
                                   by row 64 and transposes)

Device-side choices (v3; evolved from 67us -> 57us -> this):
  * q ships as fp8(e4m3); Q projects with DoubleRow fp8 matmuls
    (contraction pairs interleaved): half the Q-proj TensorE cycles and
    2MB less HBM. k stays bf16 (V accuracy needs it).
  * K^T/V^T from one fused [Wk|Wv] projection. V natural tiles come from
    HWDGE DMA-transpose (xbar), entirely off TensorE; fp8 copies for PV.
  * S^T tiles (keys on partitions) in bf16; the two tiles of a pair run
    CONCURRENTLY in the PE via row groups (tile_position (0,0)/(64,0))
    and land in one 2-bank PSUM tile, so one ScalarE exp covers both.
  * Off-diagonal (fully causal-valid) tile pairs run P @ V as ONE fp8
    DoubleRow matmul (contraction 256); diagonal tiles stay bf16 with
    triangle masks. Softmax denominators ride a ones column (row 64).
  * No on-device normalization: host divides numerator rows by l.
  * 8 dummy matmuls on a zeroed tile right after the preamble warm the
    PE clock (HAM 4/8 -> 8/8) during otherwise-dead DMA-wait time.
  * DMA: inputs on the sync HWDGE ring (k0 split fine so the first
    matmul starts ASAP; then q0, k1, q1, ...); weights on the scalar
    ring; mask consts on gpsimd SWDGE; V-transposes + output stores on
    the sync ring behind the inputs. Emission weaves next-block
    projection matmuls into the exp-paced back half of each attention
    block.
"""

import numpy as np
import ml_dtypes

B, T, C, H = 8, 2048, 1024, 64
P = 128                  # SBUF partitions
CCH = C // P             # 8 contraction chunks
G = CCH // 2             # 4 DoubleRow chunk pairs
NJ = T // P              # 16 key tiles of 128
NB = T // 512            # 4 column blocks of 512
SCALE = float(C) ** -0.5

FP8Q = True              # q fp8 + DoubleRow Q projection
FP8PV = True             # fp8 DoubleRow P@V for off-diagonal pairs
SPAIR = False            # row-paired S matmuls: the operand-dup DMA latency
                         # on the critical path costs more than the PE wins
SPAIR_DUP = True         # operands for row group h1 from duplicated tiles
VT_DMA = False           # V transposes via DMA xbar instead of TensorE
WARMUP = 8               # dummy matmuls to warm the PE clock

_cached = {}


def _build(fp8q=FP8Q, fp8pv=FP8PV, spair=SPAIR, spair_dup=SPAIR_DUP,
           vt_dma=VT_DMA, warmup=WARMUP):
    import concourse.bass as bass
    import concourse.mybir as mybir
    import concourse.tile as tile
    from concourse import bacc

    dt = mybir.dt
    nc = bacc.Bacc("TRN2", target_bir_lowering=False, debug=False, num_devices=B)

    kT = nc.dram_tensor("kT", [NB, P, CCH, 512], dt.bfloat16, kind="ExternalInput").ap()
    if fp8q:
        qT = nc.dram_tensor("qT", [NB, P, G, 2, 512], dt.float8e4,
                            kind="ExternalInput").ap()
        wq = nc.dram_tensor("wq", [P, G, 2, H], dt.float8e4,
                            kind="ExternalInput").ap()
    else:
        qT = nc.dram_tensor("qT", [NB, P, CCH, 512], dt.bfloat16,
                            kind="ExternalInput").ap()
        wq = nc.dram_tensor("wq", [P, CCH, H], dt.bfloat16,
                            kind="ExternalInput").ap()
    wkv = nc.dram_tensor("wkv", [P, CCH, 2 * H], dt.bfloat16, kind="ExternalInput").ap()
    dmask = nc.dram_tensor("dmask", [P, P], dt.bfloat16, kind="ExternalInput").ap()
    idb = nc.dram_tensor("idb", [P, P], dt.bfloat16, kind="ExternalInput").ap()
    out_t = nc.dram_tensor("out_t", [H + 1, T], dt.float32, kind="ExternalOutput").ap()

    EXP = mybir.ActivationFunctionType.Exp
    DR = mybir.MatmulPerfMode.DoubleRow

    with tile.TileContext(nc) as tc:
        with (
            tc.tile_pool(name="consts", bufs=1) as consts,
            tc.tile_pool(name="inbuf", bufs=1) as inbuf,
            tc.tile_pool(name="proj", bufs=1) as proj,
            tc.tile_pool(name="ppsum", bufs=1, space="PSUM") as ppsum,
            tc.tile_pool(name="opsum", bufs=2 if vt_dma else 1,
                         space="PSUM") as opsum,
            tc.tile_pool(name="spsum", bufs=2, space="PSUM") as spsum,
            tc.tile_pool(name="vtpsum", bufs=1, space="PSUM") as vtpsum,
            tc.tile_pool(name="pbuf", bufs=2) as pbuf,
            tc.tile_pool(name="obuf", bufs=2) as obuf,
        ):

            # ---- constants: weights on scalar HWDGE ring (earliest need),
            #      mask/identity on the gpsimd SWDGE ring -------------------
            wkv_s = consts.tile([P, CCH, 2 * H], dt.bfloat16)
            if fp8q:
                wq_s = consts.tile([P, G, 2, H], dt.float8e4)
            else:
                wq_s = consts.tile([P, CCH, H], dt.bfloat16)
            mask_s = consts.tile([P, P], dt.bfloat16)
            idb_s = consts.tile([P, P], dt.bfloat16)
            # weights lead the sync ring (the data ring is FIFO: putting
            # everything in strict need-order beats parallel rings, whose
            # transfers fair-share SDMA bandwidth and all finish late)
            nc.sync.dma_start(out=wkv_s[:], in_=wkv[:])
            nc.sync.dma_start(out=wq_s[:], in_=wq[:])
            nc.scalar.dma_start(out=mask_s[:], in_=dmask[:])
            nc.scalar.dma_start(out=idb_s[:], in_=idb[:])

            # ---- PE clock warm-up on a zeroed tile (no DMA dependence) ----
            if warmup:
                warm_s = consts.tile([P, 512], dt.bfloat16)
                nc.vector.memset(warm_s[:], 0.0)
                wt = spsum.tile([P, 2, 512], dt.float32, tag="s")
                for _ in range(warmup):
                    nc.tensor.matmul(wt[:, 0, :], lhsT=warm_s[:, 0:P],
                                     rhs=warm_s[:], start=True, stop=True)

            kT_s = inbuf.tile([P, NB, CCH, 512], dt.bfloat16)
            if fp8q:
                qT_s = inbuf.tile([P, NB, G, 2, 512], dt.float8e4)
            else:
                qT_s = inbuf.tile([P, NB, CCH, 512], dt.bfloat16)
            KVT_s = proj.tile([P, T], dt.bfloat16)   # rows 0:64 K^T, 64:128 V^T
            QT_s = proj.tile([H, T], dt.bfloat16)
            if spair and spair_dup:
                KT2_s = proj.tile([P, T], dt.bfloat16)   # K^T dup @ rows 64:128
                QT2_s = proj.tile([P, T], dt.bfloat16)   # Q^T dup @ rows 64:128
            # V natural tiles at [:, j, 0:64], ones column at [:, j, 64]
            V1_s = proj.tile([P, NJ, 80], dt.bfloat16)
            nc.vector.memset(V1_s[:, :, 64:65], 1.0)
            if fp8pv:
                V18_s = proj.tile([P, NJ, 80], dt.float8e4)
                nc.vector.memset(V18_s[:, :, 64:65], 1.0)

            # ---- input DMAs: one FIFO ring, strict need-order ---------------
            for tb in range(NB):
                nc.sync.dma_start(out=kT_s[:, tb], in_=kT[tb])
                nc.sync.dma_start(out=qT_s[:, tb], in_=qT[tb])

            # ---- projection work-units for one 512-col block ---------------
            def proj_units(tb, first=False):
                """Returns (core_units, vt_units)."""
                sl = slice(512 * tb, 512 * (tb + 1))
                KVTp = ppsum.tile([P, 512], dt.float32, tag="kvt")
                QTp = ppsum.tile([H, 512], dt.float32, tag="qt")
                core = []

                def kv_unit(cs, KVTp=KVTp):
                    for c in cs:
                        nc.tensor.matmul(KVTp[:], lhsT=wkv_s[:, c, :],
                                         rhs=kT_s[:, tb, c, :],
                                         start=(c == 0), stop=(c == CCH - 1))
                groups = ([0, 1], [2, 3], [4, 5], [6, 7])
                for cs in groups:
                    core.append(lambda cs=cs: kv_unit(cs))

                def kv_copy(KVTp=KVTp):
                    nc.vector.tensor_copy(out=KVT_s[:, sl], in_=KVTp[:])
                    if spair and spair_dup:
                        nc.gpsimd.dma_start(out=KT2_s[64:128, sl],
                                            in_=KVT_s[0:64, sl])
                core.append(kv_copy)

                if fp8q:
                    def q_unit(QTp=QTp):
                        for g in range(G):
                            nc.tensor.matmul(QTp[:], lhsT=wq_s[:, g],
                                             rhs=qT_s[:, tb, g],
                                             start=(g == 0), stop=(g == G - 1),
                                             perf_mode=DR)
                    core.append(q_unit)
                else:
                    def q_unit(cc, QTp=QTp):
                        for c in (cc, cc + 1):
                            nc.tensor.matmul(QTp[:], lhsT=wq_s[:, c, :],
                                             rhs=qT_s[:, tb, c, :],
                                             start=(c == 0), stop=(c == CCH - 1))
                    for cc in range(0, CCH, 2):
                        core.append(lambda cc=cc: q_unit(cc))

                def q_copy(QTp=QTp):
                    nc.vector.tensor_copy(out=QT_s[:, sl], in_=QTp[:])
                    if spair and spair_dup and tb >= 1:
                        # (block 0 attends unpaired - no dup needed there)
                        nc.gpsimd.dma_start(out=QT2_s[64:128, sl],
                                            in_=QT_s[0:H, sl])
                core.append(q_copy)

                def vt_unit(j):
                    if vt_dma:
                        nc.sync.dma_start_transpose(
                            out=V1_s[:, j, 0:64],
                            in_=KVT_s[64:128, P * j:P * (j + 1)])
                        if fp8pv:
                            nc.vector.tensor_copy(out=V18_s[:, j, 0:64],
                                                  in_=V1_s[:, j, 0:64])
                    else:
                        vtp = vtpsum.tile([P, P], dt.bfloat16, tag="vt")
                        nc.tensor.transpose(vtp[:], KVT_s[:, P * j:P * (j + 1)],
                                            idb_s[:])
                        nc.vector.tensor_copy(out=V1_s[:, j, 0:64],
                                              in_=vtp[:, 64:128])
                        if fp8pv:
                            nc.vector.tensor_copy(out=V18_s[:, j, 0:64],
                                                  in_=vtp[:, 64:128])
                vt = [lambda j=4 * tb + jj: vt_unit(j) for jj in range(4)]
                return core, vt

            # ---- attention block: pairs of 128-key tiles -------------------
            def attn_block(ic, weave, early):
                ilo = 512 * ic
                OUTp = opsum.tile([H + 1, 512], dt.float32, tag="out")
                pairs = []
                for u in range(2 * ic):
                    pairs.append((2 * u, 512, 2 * u + 1, 512, False))
                pairs.append((4 * ic, 512, 4 * ic + 1, 384, True))
                pairs.append((4 * ic + 2, 256, 4 * ic + 3, 128, True))
                npairs = len(pairs)
                state = {}
                pair_ok = spair and spair_dup and ic >= 1

                def emit_S(pr):
                    jA, nA, jB, nB, diag = pr
                    Sp = spsum.tile([P, 2, 512], dt.float32, tag="s")
                    loA = max(P * jA, ilo)
                    loB = max(P * jB, ilo)
                    nc.tensor.matmul(Sp[:, 0, 0:nA],
                                     lhsT=KVT_s[0:H, P * jA:P * (jA + 1)],
                                     rhs=QT_s[:, loA:loA + nA],
                                     start=True, stop=True)
                    if pair_ok:
                        # both operands must physically sit at partitions
                        # 64:128 (walrus: weight base == row tile position,
                        # fmap base == weight base) - hence the dup tiles.
                        nc.tensor.matmul(Sp[:, 1, 0:nB],
                                         lhsT=KT2_s[64:128, P * jB:P * (jB + 1)],
                                         rhs=QT2_s[64:128, loB:loB + nB],
                                         start=True, stop=True,
                                         tile_position=(64, 0))
                    else:
                        nc.tensor.matmul(Sp[:, 1, 0:nB],
                                         lhsT=KVT_s[0:H, P * jB:P * (jB + 1)],
                                         rhs=QT_s[:, loB:loB + nB],
                                         start=True, stop=True)
                    if diag or not fp8pv:
                        Pt = pbuf.tile([P, 2, 512], dt.bfloat16, tag="pd", bufs=2)
                    else:
                        Pt = pbuf.tile([P, 2, 512], dt.float8e4, tag="pf", bufs=3)
                    nc.scalar.activation(out=Pt[:, :, 0:nA], in_=Sp[:, :, 0:nA],
                                         func=EXP, scale=SCALE)
                    if diag:
                        nc.vector.tensor_mul(Pt[:, 0, 0:P], Pt[:, 0, 0:P],
                                             mask_s[:])
                        nc.vector.tensor_mul(Pt[:, 1, 0:P], Pt[:, 1, 0:P],
                                             mask_s[:])
                    state[pr[0]] = Pt

                def emit_PV(pr, first, last):
                    jA, nA, jB, nB, diag = pr
                    Pt = state.pop(pr[0])
                    if diag or not fp8pv:
                        loA = max(P * jA, ilo) - ilo
                        loB = max(P * jB, ilo) - ilo
                        nc.tensor.matmul(OUTp[:, loA:512],
                                         lhsT=V1_s[:, jA, 0:65],
                                         rhs=Pt[:, 0, 0:nA],
                                         start=first, stop=False)
                        nc.tensor.matmul(OUTp[:, loB:512],
                                         lhsT=V1_s[:, jB, 0:65],
                                         rhs=Pt[:, 1, 0:nB],
                                         start=False, stop=last)
                    else:
                        nc.tensor.matmul(OUTp[:, 0:512],
                                         lhsT=V18_s[:, jA:jA + 2, 0:65],
                                         rhs=Pt[:, :, :],
                                         start=first, stop=last,
                                         perf_mode=DR)

                # software pipeline: S(p+1) emitted before PV(p); `early`
                # units go right after S(0); `weave` into the back half.
                widx = 0
                wstart = (npairs + 1) // 2

                def drain(tgt):
                    nonlocal widx
                    while widx < min(tgt, len(weave)):
                        weave[widx]()
                        widx += 1

                emit_S(pairs[0])
                for u in early:
                    u()
                nslots = npairs - wstart
                for i in range(1, npairs):
                    emit_S(pairs[i])
                    emit_PV(pairs[i - 1], first=(i == 1), last=False)
                    if i >= wstart and nslots > 0:
                        drain((i - wstart + 1) * len(weave) // (nslots + 1))
                emit_PV(pairs[-1], first=(npairs == 1), last=True)
                drain(len(weave))

                ot = obuf.tile([H + 1, 512], dt.float32, tag="o")
                nc.vector.tensor_copy(out=ot[:], in_=OUTp[:])
                nc.sync.dma_start(out=out_t[:, ilo:ilo + 512], in_=ot[:])

            core0, vt0 = proj_units(0, first=True)
            for u in core0:
                u()
            nxt = vt0
            for ic in range(NB):
                if ic + 1 < NB:
                    core, vt = proj_units(ic + 1)
                    weave = core + vt
                else:
                    weave = []
                attn_block(ic, weave, early=nxt)
                nxt = []

    nc.compile()
    return nc


def _get_nc():
    key = (FP8Q, FP8PV, SPAIR, SPAIR_DUP, VT_DMA, WARMUP)
    if key not in _cached:
        _cached[key] = _build(*key)
    return _cached[key]


def _block(xT):
    """[C, T] -> [NB, P, CCH, 512] so each 512-col block is contiguous."""
    return np.ascontiguousarray(
        xT.reshape(CCH, P, NB, 512).transpose(2, 1, 0, 3))


def _block8(xT):
    """[C, T] -> [NB, P, G, 2, 512]; contraction chunk pairs interleaved."""
    return np.ascontiguousarray(
        xT.reshape(G, 2, P, NB, 512).transpose(3, 2, 0, 1, 4))


def _wblock(w):
    """[C, Hw] -> [P, CCH, Hw] contiguous."""
    return np.ascontiguousarray(
        w.reshape(CCH, P, w.shape[1]).transpose(1, 0, 2))


def _host_inputs(q, k, Wq, Wk, Wv):
    bf16 = ml_dtypes.bfloat16
    fp8 = ml_dtypes.float8_e4m3
    if FP8Q:
        wq_h = np.ascontiguousarray(
            Wq.astype(fp8).reshape(G, 2, P, H).transpose(2, 0, 1, 3))
    else:
        wq_h = _wblock(Wq.astype(bf16))
    wkv_h = _wblock(np.concatenate([Wk, Wv], axis=1).astype(bf16))
    dmask_h = np.triu(np.ones((P, P), dtype=np.float32)).astype(bf16)
    idb_h = np.eye(P, dtype=np.float32).astype(bf16)
    in_maps = []
    for b in range(B):
        in_maps.append({
            "qT": _block8(q[b].T.astype(fp8)) if FP8Q
                  else _block(q[b].T.astype(bf16)),
            "kT": _block(k[b].T.astype(bf16)),
            "wq": wq_h,
            "wkv": wkv_h,
            "dmask": dmask_h,
            "idb": idb_h,
        })
    return in_maps


def _unshard(results):
    """Per-core [H+1, T] (numerator^T ; l) -> [B, T, H] normalized fp32."""
    outs = []
    for b in range(B):
        ot = results[b]["out_t"].astype(np.float32)
        outs.append((ot[0:H] / ot[H:H + 1]).T)
    return np.stack(outs).astype(np.float32)


def kernel(q, k, Wq, Wk, Wv):
    from concourse.bass_utils import run_bass_kernel_spmd

    nc = _get_nc()
    in_maps = _host_inputs(q, k, Wq, Wk, Wv)
    res = run_bass_kernel_spmd(nc, in_maps, list(range(B)))
    return _unshard(res.results)


if __name__ == "__main__":
    rng = np.random.default_rng(0)
    q = rng.standard_normal((B, T, C)).astype(np.float32)
    k = rng.standard_normal((B, T, C)).astype(np.float32)
    Wq = (rng.standard_normal((C, H)) * 0.02).astype(np.float32)
    Wk = (rng.standard_normal((C, H)) * 0.02).astype(np.float32)
    Wv = (rng.standard_normal((C, H)) * 0.02).astype(np.float32)
    o = kernel(q, k, Wq, Wk, Wv)
    print("out", o.shape, o.dtype, float(np.abs(o).max()))
